# revision 1
# baseline (speedup 1.0000x reference)
"""BertSelfAttention (B=4, S=2048, H=1024, NH=16, HD=64) on 8 Trainium2 NeuronCores.

Sharding: batch (4) x head-group (2) -> 8 cores. Core c handles batch b=c//2 and
heads [g*8, g*8+8) with g=c%2 (output channels [g*512, (g+1)*512)).

Per-core math (all on device):
  QT[ch, s] = (wq_c @ x_b^T + bq_c),  KT likewise       (channels on partitions)
  V[s, ch]  = (x_b @ wv_c^T + bv_c)                     (tokens on partitions)
  per (head h, query half ih), per key tile st (128 keys j):
      scoresT[j, i] -> [128, 1024] PSUM (2 matmuls), ping-pong buffered
      expT = exp(scoresT/8 + mask_j)   (one ACT op; mask is per-partition bias)
      ctxT[d, i] += [v_h | 1]^T-weighted expT           (fused denominator row)
  Device emits unnormalized ctxT + denom rows [8*65, 2048]; the host divides and
  transposes into [B, S, H].

Two Trainium2-specific tricks matter here:
  * Changing the matmul contraction size (K) between back-to-back matmuls costs
    ~1.6us in PE reconfiguration, so every matmul keeps K=128: Q is stored
    per-head zero-padded to 128 partitions (the other head's K rows hit zeros),
    while KT stays packed two heads per tile.
  * All PSUM lives in one pool of 4 [128, 1024] tags: QKV passes use tile
    halves as 8 accumulators, attention ping-pongs scores on tags 0/1 and ctx
    on tags 2/3 -- no pool-transition barrier or head-boundary PE stalls.

Matmuls run as float32r (full-rate fp32 with hardware rounding, ~2e-4 rel err).
"""

import os
import sys

if "/opt/trn_rl_repo" not in sys.path:
    sys.path.insert(0, "/opt/trn_rl_repo")

import numpy as np

_KERNEL_DIR = os.path.dirname(os.path.abspath(__file__))

B, S, H = 4, 2048, 1024
NH, HD = 16, 64
HPC = 8          # heads per core
CH = HPC * HD    # 512 output channels per core
CT = H // 128    # 8 contraction tiles
JT = CH // 128   # 4 channel tiles per core
ST = S // 128    # 16 token tiles
VW = HD + 1      # 65: v columns + fused ones column

_CACHE = {}


def _build():
    import concourse.bass as bass  # noqa: F401  (registers engine methods)
    import concourse.mybir as mybir
    import concourse.tile as tile
    from concourse import bacc

    F32 = mybir.dt.float32
    F32R = mybir.dt.float32r

    nc = bacc.Bacc("TRN2", target_bir_lowering=False, debug=True)

    xt = nc.dram_tensor("xt", [H, S], F32, kind="ExternalInput")        # x_b^T
    wq_t = nc.dram_tensor("wq_t", [H, CH], F32, kind="ExternalInput")   # wq_c^T
    wk_t = nc.dram_tensor("wk_t", [H, CH], F32, kind="ExternalInput")
    wv_t = nc.dram_tensor("wv_t", [H, CH], F32, kind="ExternalInput")
    bq = nc.dram_tensor("bq", [CH], F32, kind="ExternalInput")
    bk = nc.dram_tensor("bk", [CH], F32, kind="ExternalInput")
    bv = nc.dram_tensor("bv", [CH], F32, kind="ExternalInput")
    mask = nc.dram_tensor("mask", [S], F32, kind="ExternalInput")
    ones = nc.dram_tensor("ones", [512], F32, kind="ExternalInput")
    # unnormalized ctxT + denominator rows, 65 rows per head
    out = nc.dram_tensor("out", [VW * HPC, S], F32, kind="ExternalOutput")

    with tile.TileContext(nc) as tc, nc.allow_low_precision(reason="fp32r attention"):
        from contextlib import ExitStack

        with ExitStack() as outer:
            persist = outer.enter_context(tc.tile_pool(name="persist", bufs=1))
            ppool = outer.enter_context(tc.tile_pool(name="pp", bufs=1, space="PSUM"))

            # Persistent SBUF tensors
            # Q per head, zero-padded to 128 partitions (head h lives in its own
            # partition range po:po+64; the other 64 rows are zeros).
            qp_sb = [persist.tile([128, S], F32R, tag=f"qp{h}", name=f"qp{h}")
                     for h in range(HPC)]
            kt_sb = [persist.tile([128, S], F32R, tag=f"kt{j}", name=f"kt{j}")
                     for j in range(JT)]
            v_sb = persist.tile([128, ST, VW * HPC], F32R, tag="v")
            mask_sb = persist.tile([128, ST], F32, tag="mask")
            bqp = persist.tile([128, JT], F32, tag="bqp")
            bkp = persist.tile([128, JT], F32, tag="bkp")
            bv_bc = persist.tile([128, CH], F32, tag="bv_bc")
            ones8 = persist.tile([128, HPC], F32R, tag="ones8")
            zcol = persist.tile([128, 1], F32, tag="zcol")

            nc.sync.dma_start(out=mask_sb, in_=mask.rearrange("(t p) -> p t", p=128))
            nc.sync.dma_start(out=bqp, in_=bq.rearrange("(j p) -> p j", p=128))
            nc.sync.dma_start(out=bkp, in_=bk.rearrange("(j p) -> p j", p=128))
            nc.sync.dma_start(
                out=bv_bc,
                in_=bass.AP(tensor=bv, offset=0, ap=[[0, 128], [1, CH]]))
            nc.sync.dma_start(
                out=ones8,
                in_=bass.AP(tensor=ones.bitcast(F32R), offset=0,
                            ap=[[0, 128], [1, HPC]]))
            # ones columns of v (position 64 of each head block, every token tile)
            v4 = v_sb.rearrange("p t (h e) -> p t h e", e=VW)
            for t in range(ST):
                nc.vector.tensor_copy(v4[:, t, :, HD], ones8)
            # zero the unused partition half of each padded-Q tile
            nc.vector.memset(zcol, 0.0)
            for h in range(HPC):
                zo = 64 if h % 2 == 0 else 0      # rows NOT owned by head h
                zsrc = zcol[zo:zo + 64, 0:1]
                zbcast = bass.AP(tensor=zsrc.tensor, offset=zsrc.offset,
                                 ap=[zsrc.ap[0], [0, S]])
                nc.vector.tensor_copy(qp_sb[h][zo:zo + 64, :], zbcast)

            # ---------------- Phase 1: QKV projections ----------------
            with ExitStack() as ph1:
                wpool = ph1.enter_context(tc.tile_pool(name="w", bufs=1))
                xqpool = ph1.enter_context(tc.tile_pool(name="xq", bufs=10))
                xpool = ph1.enter_context(tc.tile_pool(name="x", bufs=4))

                wq_r = wq_t.rearrange("(c p) j -> c p j", p=128).bitcast(F32R)
                wk_r = wk_t.rearrange("(c p) j -> c p j", p=128).bitcast(F32R)
                wv_r = wv_t.rearrange("(c p) j -> c p j", p=128).bitcast(F32R)
                xt_r = xt.rearrange("(c p) s -> c p s", p=128).bitcast(F32R)

                # stage the full first quarter: per ct interleave x, wq, wk
                # DMAs so the first pass never runs dry; wv is deferred until
                # after the QK passes (the V pass runs last).
                x_first = []
                wq_sb, wk_sb, wv_sb = [], [], []
                for ct in range(CT):
                    x_t = xqpool.tile([128, 512], F32R, tag="xq", name=f"xqk0{ct}")
                    nc.sync.dma_start(out=x_t, in_=xt_r[ct, :, 0:512])
                    x_first.append(x_t)
                    for lst, srct, nm in ((wq_sb, wq_r, "wq"), (wk_sb, wk_r, "wk")):
                        w = wpool.tile([128, CH], F32R, tag=f"{nm}{ct}",
                                       name=f"{nm}{ct}")
                        nc.sync.dma_start(out=w, in_=srct[ct])
                        lst.append(w)

                # Combined Q+K pass over query-range quarters (x streamed once).
                # PSUM tag t{j} holds Q_j in columns 0:512 and K_j in 512:1024.
                for sq in range(4):
                    pqk = [ppool.tile([128, 1024], F32, tag=f"t{j}",
                                      name=f"pqk{sq}{j}")
                           for j in range(JT)]
                    for ct in range(CT):
                        if sq == 0:
                            x_t = x_first[ct]
                        else:
                            x_t = xqpool.tile([128, 512], F32R, tag="xq",
                                              name=f"xqk{sq}_{ct}")
                            nc.sync.dma_start(
                                out=x_t,
                                in_=xt_r[ct, :, sq * 512:(sq + 1) * 512])
                        for j in range(JT):
                            nc.tensor.matmul(
                                pqk[j][:, 0:512],
                                lhsT=wq_sb[ct][:, j * 128:(j + 1) * 128],
                                rhs=x_t,
                                start=(ct == 0), stop=(ct == CT - 1))
                        for j in range(JT):
                            nc.tensor.matmul(
                                pqk[j][:, 512:1024],
                                lhsT=wk_sb[ct][:, j * 128:(j + 1) * 128],
                                rhs=x_t,
                                start=(ct == 0), stop=(ct == CT - 1))
                    for j in range(JT):
                        # drain each tag via three engines-worth of copies:
                        # q head-even on ACT, q head-odd + k on DVE
                        h0, h1 = 2 * j, 2 * j + 1
                        nc.scalar.activation(
                            qp_sb[h0][0:64, sq * 512:(sq + 1) * 512],
                            pqk[j][0:64, 0:512],
                            mybir.ActivationFunctionType.Identity,
                            bias=bqp[0:64, j:j + 1], scale=1.0)
                        nc.vector.tensor_scalar_add(
                            qp_sb[h1][64:128, sq * 512:(sq + 1) * 512],
                            pqk[j][64:128, 0:512],
                            bqp[64:128, j:j + 1])
                        nc.vector.tensor_scalar_add(
                            kt_sb[j][:, sq * 512:(sq + 1) * 512],
                            pqk[j][:, 512:1024],
                            bkp[:, j:j + 1])

                # V pass: tokens on psum partitions (x streamed a second time).
                for ct in range(CT):
                    w = wpool.tile([128, CH], F32R, tag=f"wq{ct}", name=f"wv{ct}")
                    nc.sync.dma_start(out=w, in_=wv_r[ct])
                    wv_sb.append(w)
                for sh in range(2):
                    pv = [ppool.tile([128, 1024], F32, tag=f"t{j}",
                                     name=f"pv{sh}{j}")
                          for j in range(JT)]
                    for ct in range(CT):
                        x_t = xpool.tile([128, 1024], F32R, tag="x",
                                         name=f"xv{sh}{ct}")
                        nc.sync.dma_start(
                            out=x_t, in_=xt_r[ct, :, sh * 1024:(sh + 1) * 1024])
                        for st in range(8):
                            nc.tensor.matmul(
                                pv[st // 2][:, (st % 2) * 512:(st % 2 + 1) * 512],
                                lhsT=x_t[:, st * 128:(st + 1) * 128],
                                rhs=wv_sb[ct],
                                start=(ct == 0), stop=(ct == CT - 1))
                    for st in range(8):
                        sl = pv[st // 2][:, (st % 2) * 512:(st % 2 + 1) * 512]
                        for h in range(HPC):
                            nc.vector.tensor_add(
                                v_sb[:, sh * 8 + st, h * VW:h * VW + HD],
                                sl[:, h * HD:(h + 1) * HD],
                                bv_bc[:, h * HD:(h + 1) * HD])

            # ---------------- Phase 2: attention ----------------
            with ExitStack() as ph2:
                epool = ph2.enter_context(tc.tile_pool(name="ep", bufs=8))
                opool = ph2.enter_context(tc.tile_pool(name="op", bufs=3))

                for h in range(HPC):
                    qi = h // 2
                    for ih in range(2):
                        blk = h * 2 + ih
                        i0 = ih * 1024
                        ctx_ps = ppool.tile([VW, 1024], F32, tag=f"t{2 + blk % 2}",
                                            name=f"ctx{blk}")
                        for st in range(ST):
                            s_ps = ppool.tile([128, 1024], F32, tag=f"t{st % 2}",
                                              name=f"sc{blk}_{st}")
                            for q in range(2):
                                nc.tensor.matmul(
                                    s_ps[:, q * 512:(q + 1) * 512],
                                    lhsT=kt_sb[qi][:, st * 128:(st + 1) * 128],
                                    rhs=qp_sb[h][:, i0 + q * 512:i0 + (q + 1) * 512],
                                    start=True, stop=True)
                            e_sb = epool.tile([128, 1024], F32R, tag="e",
                                              name=f"e{blk}_{st}")
                            nc.scalar.activation(
                                e_sb, s_ps,
                                mybir.ActivationFunctionType.Exp,
                                bias=mask_sb[:, st:st + 1], scale=0.125)
                            for q in range(2):
                                nc.tensor.matmul(
                                    ctx_ps[:, q * 512:(q + 1) * 512],
                                    lhsT=v_sb[:, st, h * VW:(h + 1) * VW],
                                    rhs=e_sb[:, q * 512:(q + 1) * 512],
                                    start=(st == 0), stop=(st == ST - 1))
                        o_sb = opool.tile([VW, 1024], F32, tag="o", name=f"o{blk}")
                        nc.vector.tensor_copy(o_sb, ctx_ps)
                        nc.sync.dma_start(
                            out=out[h * VW:(h + 1) * VW, i0:i0 + 1024], in_=o_sb)

    nc.compile()
    return nc


def _get_nc():
    if "nc" not in _CACHE:
        _CACHE["nc"] = _build()
    return _CACHE["nc"]


def _in_maps(hidden_states, attention_mask, wq, bq, wk, bk, wv, bv):
    ones = np.ones(512, np.float32)
    maps = []
    for c in range(8):
        b, g = c // 2, c % 2
        ch0 = g * CH
        maps.append({
            "xt": np.ascontiguousarray(hidden_states[b].T),
            "wq_t": np.ascontiguousarray(wq[ch0:ch0 + CH, :].T),
            "wk_t": np.ascontiguousarray(wk[ch0:ch0 + CH, :].T),
            "wv_t": np.ascontiguousarray(wv[ch0:ch0 + CH, :].T),
            "bq": np.ascontiguousarray(bq[ch0:ch0 + CH]),
            "bk": np.ascontiguousarray(bk[ch0:ch0 + CH]),
            "bv": np.ascontiguousarray(bv[ch0:ch0 + CH]),
            "mask": np.ascontiguousarray(attention_mask[b, 0, 0, :]),
            "ones": ones,
        })
    return maps


def _gather(results):
    full = np.empty((B, S, H), np.float32)
    for c in range(8):
        b, g = c // 2, c % 2
        o = results[c]["out"].reshape(HPC, VW, S)
        ctx = o[:, :HD, :] / o[:, HD:HD + 1, :]        # normalize by denom row
        # [h, d, s] -> [s, h*d]
        full[b, :, g * CH:(g + 1) * CH] = ctx.reshape(CH, S).T
    return full


def _run(in_maps, trace=False):
    from concourse.bass_utils import run_bass_kernel_spmd

    nc = _get_nc()
    return run_bass_kernel_spmd(nc, in_maps, list(range(8)), trace=trace)


def _run_results(in_maps):
    """Run on hardware; on a wedged-device error retry in fresh subprocesses
    (the PJRT client cannot recover an unrecoverable exec unit in-process)."""
    try:
        return _run(in_maps).results
    except Exception:
        pass
    import pickle
    import subprocess
    import tempfile

    last = None
    for _ in range(3):
        try:
            with tempfile.TemporaryDirectory() as td:
                fin = os.path.join(td, "in.pkl")
                fout = os.path.join(td, "out.pkl")
                with open(fin, "wb") as f:
                    pickle.dump(in_maps, f)
                code = (
                    "import pickle, sys\n"
                    f"sys.path.insert(0, {_KERNEL_DIR!r})\n"
                    "import kernel\n"
                    f"maps = pickle.load(open({fin!r}, 'rb'))\n"
                    "res = kernel._run(maps)\n"
                    f"pickle.dump(res.results, open({fout!r}, 'wb'))\n"
                )
                subprocess.run([sys.executable, "-c", code], check=True,
                               timeout=1800)
                with open(fout, "rb") as f:
                    return pickle.load(f)
        except Exception as e:
            last = e
    raise last


def kernel(hidden_states, attention_mask, wq, bq, wk, bk, wv, bv):
    args = [np.asarray(a, np.float32) for a in
            (hidden_states, attention_mask, wq, bq, wk, bk, wv, bv)]
    return _gather(_run_results(_in_maps(*args)))


def kernel_profiled(hidden_states, attention_mask, wq, bq, wk, bk, wv, bv):
    """Like kernel() but with NTFF tracing; returns (output, exec_time_ns)."""
    args = [np.asarray(a, np.float32) for a in
            (hidden_states, attention_mask, wq, bq, wk, bk, wv, bv)]
    res = _run(_in_maps(*args), trace=True)
    return _gather(res.results), res.exec_time_ns



# revision 3
# speedup vs baseline: 1.1723x; 1.1723x over previous
"""BertSelfAttention (B=4, S=2048, H=1024, NH=16, HD=64) on 8 Trainium2 NeuronCores.

Sharding: batch (4) x head-group (2) -> 8 cores. Core c handles batch b=c//2 and
heads [g*8, g*8+8) with g=c%2 (output channels [g*512, (g+1)*512)).

v2: single interleaved instruction stream, bf16 matmul path.

The kernel is ACT-bound: softmax exp is 33.6M elements/core and only the
Scalar engine evaluates Exp (1 elem/cycle/lane @ 1.2 GHz) -> ~294us minimum.
Everything else is scheduled around keeping ACT busy from ~7us onward:

  * All matmul inputs are bf16 (host casts). This turns on Fast Weight Load
    so LDWEIGHTS hides behind the previous matmul's streaming, halves DMA
    and SBUF, and keeps PE at ~N/2.4GHz per matmul. PSUM stays fp32.
  * x^T lives in SBUF for the whole kernel (4MB bf16, 32 [128,512] tiles),
    so the Q/K and V passes never re-stream it from HBM.
  * QKV projection is emitted as 1-PSUM-bank chunks (8 matmuls + 1-2 DVE
    drain ops each) interleaved into the attention loop: only Q(queries
    0:1024), K(keys 0:512) and V(tokens 0:512) are computed up front
    (~12us), then each attention iteration pops due background chunks so
    the PE's per-iteration slack (ACT 1147ns vs PE ~880ns) absorbs the
    remaining ~70us of projection work.
  * PSUM = 8 banks exactly: scores ping-pong 2x[128,1024] (4 banks), one
    ctx accumulator [65,1024] (2), projection chunk ping-pong 2x[128,512]
    (2). The ctx accumulator is single-buffered; its end-of-block drain
    stall is absorbed by background chunks.
  * Q is stored per-head zero-padded to 128 partitions (as in v1) so the
    scores contraction is always K=128 against the 2-head-packed K tiles.
  * The fused ones column in V (65 cols/head) makes the softmax denominator
    a free extra output row of the ctx matmul; the host divides.

Device emits unnormalized ctxT + denom rows [8*65, 2048] fp32; the host
divides and transposes into [B, S, H].
"""

import os
import sys

if "/opt/trn_rl_repo" not in sys.path:
    sys.path.insert(0, "/opt/trn_rl_repo")

import numpy as np

_KERNEL_DIR = os.path.dirname(os.path.abspath(__file__))

B, S, H = 4, 2048, 1024
NH, HD = 16, 64
HPC = 8          # heads per core
CH = HPC * HD    # 512 output channels per core
CT = H // 128    # 8 contraction tiles
ST = S // 128    # 16 key/token tiles
VW = HD + 1      # 65: v columns + fused ones column

_CACHE = {}


def _build():
    import concourse.bass as bass  # noqa: F401  (registers engine methods)
    import concourse.mybir as mybir
    import concourse.tile as tile
    from concourse import bacc
    from contextlib import ExitStack

    F32 = mybir.dt.float32
    BF16 = mybir.dt.bfloat16

    nc = bacc.Bacc("TRN2", target_bir_lowering=False, debug=True)

    xt = nc.dram_tensor("xt", [H, S], BF16, kind="ExternalInput")        # x_b^T
    wq_t = nc.dram_tensor("wq_t", [H, CH], BF16, kind="ExternalInput")   # wq_c^T
    wk_t = nc.dram_tensor("wk_t", [H, CH], BF16, kind="ExternalInput")
    wv_t = nc.dram_tensor("wv_t", [H, CH], BF16, kind="ExternalInput")
    bq = nc.dram_tensor("bq", [CH], F32, kind="ExternalInput")
    bk = nc.dram_tensor("bk", [CH], F32, kind="ExternalInput")
    bv = nc.dram_tensor("bv", [CH], F32, kind="ExternalInput")
    mask = nc.dram_tensor("mask", [S], F32, kind="ExternalInput")
    # unnormalized ctxT + denominator rows, 65 rows per head
    out = nc.dram_tensor("out", [VW * HPC, S], F32, kind="ExternalOutput")

    with tile.TileContext(nc) as tc, nc.allow_low_precision(reason="bf16 attention"):
        with ExitStack() as stk:
            persist = stk.enter_context(tc.tile_pool(name="persist", bufs=1))
            ppool = stk.enter_context(tc.tile_pool(name="pp", bufs=1, space="PSUM"))
            epool = stk.enter_context(tc.tile_pool(name="ep", bufs=8))
            opool = stk.enter_context(tc.tile_pool(name="op", bufs=3))

            xt_r = xt.rearrange("(c p) s -> c p s", p=128)
            wq_r = wq_t.rearrange("(c p) j -> c p j", p=128)
            wk_r = wk_t.rearrange("(c p) j -> c p j", p=128)
            wv_r = wv_t.rearrange("(c p) j -> c p j", p=128)

            # ---- persistent SBUF tensors ----
            # x^T resident: 32 tiles [128, 512] (ct x query-quarter)
            x_sb = [[persist.tile([128, 512], BF16, tag=f"x{ct}_{sq}",
                                  name=f"x{ct}_{sq}")
                     for sq in range(4)] for ct in range(CT)]
            wq_sb = [persist.tile([128, CH], BF16, tag=f"wq{ct}", name=f"wq{ct}")
                     for ct in range(CT)]
            wk_sb = [persist.tile([128, CH], BF16, tag=f"wk{ct}", name=f"wk{ct}")
                     for ct in range(CT)]
            wv_sb = [persist.tile([128, CH], BF16, tag=f"wv{ct}", name=f"wv{ct}")
                     for ct in range(CT)]
            # Q per head, zero-padded to 128 partitions (head h owns rows
            # (h%2)*64:(h%2)*64+64; the other 64 rows are zeros).
            qp_sb = [persist.tile([128, S], BF16, tag=f"qp{h}", name=f"qp{h}")
                     for h in range(HPC)]
            kt_sb = [persist.tile([128, S], BF16, tag=f"kt{j}", name=f"kt{j}")
                     for j in range(4)]
            v_sb = persist.tile([128, ST, VW * HPC], BF16, tag="v")
            mask_sb = persist.tile([128, ST], F32, tag="mask")
            bqp = persist.tile([128, 4], F32, tag="bqp")
            bkp = persist.tile([128, 4], F32, tag="bkp")
            bv_bc = persist.tile([128, CH], F32, tag="bv_bc")

            # ---- staging DMAs: first-chunk needs lead ----
            for ct in range(CT):
                nc.sync.dma_start(out=wq_sb[ct], in_=wq_r[ct])
                nc.sync.dma_start(out=wk_sb[ct], in_=wk_r[ct])
                nc.sync.dma_start(out=x_sb[ct][0],
                                  in_=xt_r[ct, :, 0:512])
            for ct in range(CT):
                nc.sync.dma_start(out=wv_sb[ct], in_=wv_r[ct])
            for sq in range(1, 4):
                for ct in range(CT):
                    nc.sync.dma_start(out=x_sb[ct][sq],
                                      in_=xt_r[ct, :, sq * 512:(sq + 1) * 512])
            nc.sync.dma_start(out=mask_sb, in_=mask.rearrange("(t p) -> p t", p=128))
            nc.sync.dma_start(out=bqp, in_=bq.rearrange("(j p) -> p j", p=128))
            nc.sync.dma_start(out=bkp, in_=bk.rearrange("(j p) -> p j", p=128))
            nc.sync.dma_start(
                out=bv_bc,
                in_=bass.AP(tensor=bv, offset=0, ap=[[0, 128], [1, CH]]))

            # zero the unused partition half of each padded-Q tile, and set
            # the fused ones columns of v (position 64 of each head block)
            for h in range(HPC):
                zo = 64 if h % 2 == 0 else 0      # rows NOT owned by head h
                nc.vector.memset(qp_sb[h][zo:zo + 64, :], 0.0)
            v4 = v_sb.rearrange("p t (h e) -> p t h e", e=VW)
            nc.vector.memset(v4[:, :, :, HD], 1.0)

            # ---- background projection chunks ----
            # Each chunk: 8 accumulating matmuls into one [128,512] PSUM bank
            # + drain op(s). Ping-pong across tags P0/P1.
            pstate = {"n": 0}

            def pchunk(nm):
                t = ppool.tile([128, 512], F32, tag=f"P{pstate['n'] % 2}",
                               name=nm)
                pstate["n"] += 1
                return t

            def emit_q(j, sq):
                p = pchunk(f"pq{j}_{sq}")
                for ct in range(CT):
                    nc.tensor.matmul(
                        p, lhsT=wq_sb[ct][:, j * 128:(j + 1) * 128],
                        rhs=x_sb[ct][sq],
                        start=(ct == 0), stop=(ct == CT - 1))
                h0, h1 = 2 * j, 2 * j + 1
                qr = slice(sq * 512, (sq + 1) * 512)
                nc.vector.tensor_scalar_add(
                    qp_sb[h0][0:64, qr], p[0:64, :], bqp[0:64, j:j + 1])
                nc.vector.tensor_scalar_add(
                    qp_sb[h1][64:128, qr], p[64:128, :], bqp[64:128, j:j + 1])

            def emit_k(j, sq):
                p = pchunk(f"pk{j}_{sq}")
                for ct in range(CT):
                    nc.tensor.matmul(
                        p, lhsT=wk_sb[ct][:, j * 128:(j + 1) * 128],
                        rhs=x_sb[ct][sq],
                        start=(ct == 0), stop=(ct == CT - 1))
                nc.vector.tensor_scalar_add(
                    kt_sb[j][:, sq * 512:(sq + 1) * 512], p, bkp[:, j:j + 1])

            def emit_v(t):
                p = pchunk(f"pv{t}")
                sq, c0 = t // 4, (t % 4) * 128
                for ct in range(CT):
                    nc.tensor.matmul(
                        p, lhsT=x_sb[ct][sq][:, c0:c0 + 128], rhs=wv_sb[ct],
                        start=(ct == 0), stop=(ct == CT - 1))
                # strided write skips each head's ones column
                vt = v4[:, t, :, 0:HD]
                nc.vector.tensor_add(vt, p.rearrange("p (h e) -> p h e", e=HD),
                                     bv_bc.rearrange("p (h e) -> p h e", e=HD))

            # upfront: enough for attention block (h0, ih0) to start
            emit_q(0, 0)
            emit_q(0, 1)
            emit_k(0, 0)
            for t in range(4):
                emit_v(t)

            # background queue: (due_iter, thunk). due_iter is the global
            # attention iteration (0..255) by which the chunk must be done;
            # emitted ~2 iterations earlier than strictly needed.
            bgq = []

            def bg(due, fn, *a):
                bgq.append((due, fn, a))

            # j0 remainder. NOTE: due=D means "emitted at the end of
            # iteration D-1" and trace order is semantic order, so a chunk
            # whose output iteration-D consumes MUST have due <= D:
            # V(t) feeds ctx at iter t, K(j0,q) feeds scores at iter 4q.
            bg(2, emit_k, 0, 1)
            for t in range(4, 16):
                bg(t, emit_v, t)
            bg(7, emit_k, 0, 2)
            bg(11, emit_k, 0, 3)
            bg(13, emit_q, 0, 2)   # block 1 = (h0, ih1) starts at iter 16
            bg(14, emit_q, 0, 3)
            # j1..j3: heads 2j start at iter 64*j
            for j in range(1, 4):
                base = 64 * j - 24
                bg(base + 0, emit_q, j, 0)
                bg(base + 3, emit_q, j, 1)
                bg(base + 6, emit_k, j, 0)
                bg(base + 9, emit_k, j, 1)
                bg(base + 12, emit_k, j, 2)
                bg(base + 15, emit_k, j, 3)
                bg(base + 18, emit_q, j, 2)
                bg(base + 21, emit_q, j, 3)
            bgq.sort(key=lambda x: x[0])
            bgi = {"i": 0}

            def drain_due(it):
                while bgi["i"] < len(bgq) and bgq[bgi["i"]][0] <= it:
                    _, fn, a = bgq[bgi["i"]]
                    bgi["i"] += 1
                    fn(*a)

            # ---- attention ----
            it = 0
            for h in range(HPC):
                j = h // 2
                for ih in range(2):
                    i0 = ih * 1024
                    ctx = ppool.tile([VW, 1024], F32, tag="C",
                                     name=f"ctx{h}_{ih}")
                    for st in range(ST):
                        s_ps = ppool.tile([128, 1024], F32, tag=f"S{st % 2}",
                                          name=f"sc{h}_{ih}_{st}")
                        for q2 in range(2):
                            nc.tensor.matmul(
                                s_ps[:, q2 * 512:(q2 + 1) * 512],
                                lhsT=kt_sb[j][:, st * 128:(st + 1) * 128],
                                rhs=qp_sb[h][:, i0 + q2 * 512:i0 + (q2 + 1) * 512],
                                start=True, stop=True)
                        e_sb = epool.tile([128, 1024], BF16, tag="e",
                                          name=f"e{h}_{ih}_{st}")
                        nc.scalar.activation(
                            e_sb, s_ps,
                            mybir.ActivationFunctionType.Exp,
                            bias=mask_sb[:, st:st + 1], scale=0.125)
                        for q2 in range(2):
                            nc.tensor.matmul(
                                ctx[:, q2 * 512:(q2 + 1) * 512],
                                lhsT=v_sb[:, st, h * VW:(h + 1) * VW],
                                rhs=e_sb[:, q2 * 512:(q2 + 1) * 512],
                                start=(st == 0), stop=(st == ST - 1))
                        it += 1
                        drain_due(it)
                    o_sb = opool.tile([VW, 1024], F32, tag="o",
                                      name=f"o{h}_{ih}")
                    nc.vector.tensor_copy(o_sb, ctx)
                    nc.sync.dma_start(
                        out=out[h * VW:(h + 1) * VW, i0:i0 + 1024], in_=o_sb)

    nc.compile()
    return nc


def _get_nc():
    if "nc" not in _CACHE:
        _CACHE["nc"] = _build()
    return _CACHE["nc"]


def _in_maps(hidden_states, attention_mask, wq, bq, wk, bk, wv, bv):
    import ml_dtypes

    bf16 = ml_dtypes.bfloat16
    maps = []
    for c in range(8):
        b, g = c // 2, c % 2
        ch0 = g * CH
        maps.append({
            "xt": np.ascontiguousarray(hidden_states[b].T.astype(bf16)),
            "wq_t": np.ascontiguousarray(wq[ch0:ch0 + CH, :].T.astype(bf16)),
            "wk_t": np.ascontiguousarray(wk[ch0:ch0 + CH, :].T.astype(bf16)),
            "wv_t": np.ascontiguousarray(wv[ch0:ch0 + CH, :].T.astype(bf16)),
            "bq": np.ascontiguousarray(bq[ch0:ch0 + CH]),
            "bk": np.ascontiguousarray(bk[ch0:ch0 + CH]),
            "bv": np.ascontiguousarray(bv[ch0:ch0 + CH]),
            "mask": np.ascontiguousarray(attention_mask[b, 0, 0, :]),
        })
    return maps


def _gather(results):
    full = np.empty((B, S, H), np.float32)
    for c in range(8):
        b, g = c // 2, c % 2
        o = results[c]["out"].reshape(HPC, VW, S)
        ctx = o[:, :HD, :] / o[:, HD:HD + 1, :]        # normalize by denom row
        # [h, d, s] -> [s, h*d]
        full[b, :, g * CH:(g + 1) * CH] = ctx.reshape(CH, S).T
    return full


def _run(in_maps, trace=False):
    from concourse.bass_utils import run_bass_kernel_spmd

    nc = _get_nc()
    return run_bass_kernel_spmd(nc, in_maps, list(range(8)), trace=trace)


def _run_results(in_maps):
    """Run on hardware; on a wedged-device error retry in fresh subprocesses
    (the PJRT client cannot recover an unrecoverable exec unit in-process)."""
    try:
        return _run(in_maps).results
    except Exception:
        pass
    import pickle
    import subprocess
    import tempfile

    last = None
    for _ in range(3):
        try:
            with tempfile.TemporaryDirectory() as td:
                fin = os.path.join(td, "in.pkl")
                fout = os.path.join(td, "out.pkl")
                with open(fin, "wb") as f:
                    pickle.dump(in_maps, f)
                code = (
                    "import pickle, sys\n"
                    f"sys.path.insert(0, {_KERNEL_DIR!r})\n"
                    "import kernel\n"
                    f"maps = pickle.load(open({fin!r}, 'rb'))\n"
                    "res = kernel._run(maps)\n"
                    f"pickle.dump(res.results, open({fout!r}, 'wb'))\n"
                )
                subprocess.run([sys.executable, "-c", code], check=True,
                               timeout=1800)
                with open(fout, "rb") as f:
                    return pickle.load(f)
        except Exception as e:
            last = e
    raise last


def kernel(hidden_states, attention_mask, wq, bq, wk, bk, wv, bv):
    args = [np.asarray(a, np.float32) for a in
            (hidden_states, attention_mask, wq, bq, wk, bk, wv, bv)]
    return _gather(_run_results(_in_maps(*args)))


def kernel_profiled(hidden_states, attention_mask, wq, bq, wk, bk, wv, bv):
    """Like kernel() but with NTFF tracing; returns (output, exec_time_ns)."""
    args = [np.asarray(a, np.float32) for a in
            (hidden_states, attention_mask, wq, bq, wk, bk, wv, bv)]
    res = _run(_in_maps(*args), trace=True)
    return _gather(res.results), res.exec_time_ns


# revision 4
# speedup vs baseline: 1.3136x; 1.1205x over previous
"""BertSelfAttention (B=4, S=2048, H=1024, NH=16, HD=64) on 8 Trainium2 NeuronCores.

Sharding: batch (4) x head-group (2) -> 8 cores. Core c handles batch b=c//2 and
heads [g*8, g*8+8) with g=c%2 (output channels [g*512, (g+1)*512)).

v3: bf16 matmul path, software-pipelined two-stream emission.

The kernel sits at a balanced machine point: softmax exp is 33.6M
elements/core on the only engine that can evaluate Exp (ScalarE, 1
elem/cycle/lane @ 1.2 GHz -> ~294us), while the PE streams 720896 matmul
columns (~300us @ 2.4 GHz). Span is therefore set by whichever engine
stalls less, and the whole design is about keeping both >95% busy:

  * All matmul inputs are bf16 (host casts): Fast Weight Load hides
    LDWEIGHTS behind the previous matmul's streaming; DMA and SBUF halve.
  * x^T stays resident in SBUF (4MB bf16), loaded in 8 batched DMAs; the
    4 scalar staging DMAs (mask/biases) are emitted first so nothing
    waits on them. 18 dma_starts total (~1us SWDGE setup each).
  * Two-stream software pipeline over the 256 attention iterations:
    the FRONT stream (scores matmuls + exp) runs ahead; the BACK stream
    (ctx matmuls, V-projection chunks, ctx drains) trails LAG=12
    iterations through a 16-deep bf16 e-tile pool. The ACT engine only
    ever waits on scores; V-projection storms and ctx drains are
    absorbed by the lag.
  * QKV projection is emitted as 1-PSUM-bank chunks (8 matmuls + DVE
    drain): Q/K chunks are due-scheduled into the front stream just
    before the scores that read them, V chunks into the back stream just
    before the ctx that reads them, paced so the PE's per-iteration
    slack absorbs them.
  * PSUM = 8 banks: scores ping-pong 2x[128,1024] (4), one ctx
    accumulator [65,1024] (2), projection chunk ping-pong 2x[128,512] (2).
  * Q is stored per-head zero-padded to 128 partitions so the scores
    contraction is always K=128 against 2-head-packed K tiles; the fused
    ones column in V (65 cols/head) makes the softmax denominator a free
    extra ctx output row; the host divides.

Device emits unnormalized ctxT + denom rows [8*65, 2048] fp32; the host
divides and transposes into [B, S, H].
"""

import os
import sys

if "/opt/trn_rl_repo" not in sys.path:
    sys.path.insert(0, "/opt/trn_rl_repo")

import numpy as np

_KERNEL_DIR = os.path.dirname(os.path.abspath(__file__))

B, S, H = 4, 2048, 1024
NH, HD = 16, 64
HPC = 8          # heads per core
CH = HPC * HD    # 512 output channels per core
CT = H // 128    # 8 contraction tiles
ST = S // 128    # 16 key/token tiles
VW = HD + 1      # 65: v columns + fused ones column
LAG = 12         # back-stream (ctx) lag in iterations
EBUFS = 16       # e-tile pool depth (must be > LAG + 2)

_CACHE = {}


def _build():
    import concourse.bass as bass  # noqa: F401  (registers engine methods)
    import concourse.mybir as mybir
    import concourse.tile as tile
    from concourse import bacc
    from contextlib import ExitStack

    F32 = mybir.dt.float32
    BF16 = mybir.dt.bfloat16

    nc = bacc.Bacc("TRN2", target_bir_lowering=False, debug=True)

    xt = nc.dram_tensor("xt", [H, S], BF16, kind="ExternalInput")        # x_b^T
    wq_t = nc.dram_tensor("wq_t", [H, CH], BF16, kind="ExternalInput")   # wq_c^T
    wk_t = nc.dram_tensor("wk_t", [H, CH], BF16, kind="ExternalInput")
    wv_t = nc.dram_tensor("wv_t", [H, CH], BF16, kind="ExternalInput")
    bq = nc.dram_tensor("bq", [CH], F32, kind="ExternalInput")
    bk = nc.dram_tensor("bk", [CH], F32, kind="ExternalInput")
    bv = nc.dram_tensor("bv", [CH], F32, kind="ExternalInput")
    mask = nc.dram_tensor("mask", [S], F32, kind="ExternalInput")
    # unnormalized ctxT + denominator rows, 65 rows per head
    out = nc.dram_tensor("out", [VW * HPC, S], F32, kind="ExternalOutput")

    with tile.TileContext(nc) as tc, nc.allow_low_precision(reason="bf16 attention"):
        with ExitStack() as stk:
            persist = stk.enter_context(tc.tile_pool(name="persist", bufs=1))
            ppool = stk.enter_context(tc.tile_pool(name="pp", bufs=1, space="PSUM"))
            epool = stk.enter_context(tc.tile_pool(name="ep", bufs=EBUFS))
            opool = stk.enter_context(tc.tile_pool(name="op", bufs=3))

            # ---- persistent SBUF tensors ----
            # x^T resident: 4 tiles [128, 8*512], one per query-quarter sq;
            # ct block ct lives at cols [ct*512, (ct+1)*512).
            x_sb = [persist.tile([128, CT * 512], BF16, tag=f"x{sq}",
                                 name=f"x{sq}") for sq in range(4)]
            # weights: one tile per tensor, ct block at cols [ct*512, ...)
            wq_sb = persist.tile([128, CT * 512], BF16, tag="wq")
            wk_sb = persist.tile([128, CT * 512], BF16, tag="wk")
            wv_sb = persist.tile([128, CT * 512], BF16, tag="wv")
            # Q per head, zero-padded to 128 partitions (head h owns rows
            # (h%2)*64:(h%2)*64+64; the other 64 rows are zeros).
            qp_sb = [persist.tile([128, S], BF16, tag=f"qp{h}", name=f"qp{h}")
                     for h in range(HPC)]
            kt_sb = [persist.tile([128, S], BF16, tag=f"kt{j}", name=f"kt{j}")
                     for j in range(4)]
            v_sb = persist.tile([128, ST, VW * HPC], BF16, tag="v")
            mask_sb = persist.tile([128, ST], F32, tag="mask")
            bqp = persist.tile([128, 4], F32, tag="bqp")
            bkp = persist.tile([128, 4], F32, tag="bkp")
            bv_bc = persist.tile([128, CH], F32, tag="bv_bc")

            # ---- zero-pad memsets first (DVE is idle early) ----
            for h in range(HPC):
                zo = 64 if h % 2 == 0 else 0      # rows NOT owned by head h
                nc.vector.memset(qp_sb[h][zo:zo + 64, :], 0.0)
            v4 = v_sb.rearrange("p t (h e) -> p t h e", e=VW)
            nc.vector.memset(v4[:, :, :, HD], 1.0)

            # ---- staging DMAs: scalars first, then batched x/w ----
            nc.sync.dma_start(out=mask_sb, in_=mask.rearrange("(t p) -> p t", p=128))
            nc.sync.dma_start(out=bqp, in_=bq.rearrange("(j p) -> p j", p=128))
            nc.sync.dma_start(out=bkp, in_=bk.rearrange("(j p) -> p j", p=128))
            nc.sync.dma_start(
                out=bv_bc,
                in_=bass.AP(tensor=bv, offset=0, ap=[[0, 128], [1, CH]]))

            # xt [H=(c p), S=(q s)] -> [p, c, q, s]
            xt_r = xt.rearrange("(c p) (q s) -> p c q s", p=128, s=512)
            w_r = {t.name: t.rearrange("(c p) j -> p c j", p=128)
                   for t in (wq_t, wk_t, wv_t)}

            def dma_w(w_sb, dram_name, quad):
                cs = slice(4 * quad, 4 * quad + 4)
                nc.sync.dma_start(
                    out=w_sb[:, quad * 2048:(quad + 1) * 2048].rearrange(
                        "p (c j) -> p c j", j=512),
                    in_=w_r[dram_name][:, cs, :])

            def dma_x(sq, quad):
                cs = slice(4 * quad, 4 * quad + 4)
                nc.sync.dma_start(
                    out=x_sb[sq][:, quad * 2048:(quad + 1) * 2048].rearrange(
                        "p (c s) -> p c s", s=512),
                    in_=xt_r[:, cs, sq, :])

            dma_w(wq_sb, "wq_t", 0)
            dma_w(wq_sb, "wq_t", 1)
            dma_x(0, 0)
            dma_x(0, 1)
            dma_w(wk_sb, "wk_t", 0)
            dma_w(wk_sb, "wk_t", 1)
            dma_w(wv_sb, "wv_t", 0)
            dma_w(wv_sb, "wv_t", 1)
            for sq in range(1, 4):
                dma_x(sq, 0)
                dma_x(sq, 1)

            # ---- projection chunks (1 PSUM bank each) ----
            pstate = {"n": 0}

            def pchunk(nm):
                t = ppool.tile([128, 512], F32, tag=f"P{pstate['n'] % 2}",
                               name=nm)
                pstate["n"] += 1
                return t

            def emit_q(j, sq):
                p = pchunk(f"pq{j}_{sq}")
                for ct in range(CT):
                    nc.tensor.matmul(
                        p, lhsT=wq_sb[:, ct * 512 + j * 128:ct * 512 + (j + 1) * 128],
                        rhs=x_sb[sq][:, ct * 512:(ct + 1) * 512],
                        start=(ct == 0), stop=(ct == CT - 1))
                h0, h1 = 2 * j, 2 * j + 1
                qr = slice(sq * 512, (sq + 1) * 512)
                nc.vector.tensor_scalar_add(
                    qp_sb[h0][0:64, qr], p[0:64, :], bqp[0:64, j:j + 1])
                nc.vector.tensor_scalar_add(
                    qp_sb[h1][64:128, qr], p[64:128, :], bqp[64:128, j:j + 1])

            def emit_k(j, sq):
                p = pchunk(f"pk{j}_{sq}")
                for ct in range(CT):
                    nc.tensor.matmul(
                        p, lhsT=wk_sb[:, ct * 512 + j * 128:ct * 512 + (j + 1) * 128],
                        rhs=x_sb[sq][:, ct * 512:(ct + 1) * 512],
                        start=(ct == 0), stop=(ct == CT - 1))
                nc.vector.tensor_scalar_add(
                    kt_sb[j][:, sq * 512:(sq + 1) * 512], p, bkp[:, j:j + 1])

            def emit_v(t):
                p = pchunk(f"pv{t}")
                sq, c0 = t // 4, (t % 4) * 128
                for ct in range(CT):
                    nc.tensor.matmul(
                        p, lhsT=x_sb[sq][:, ct * 512 + c0:ct * 512 + c0 + 128],
                        rhs=wv_sb[:, ct * 512:(ct + 1) * 512],
                        start=(ct == 0), stop=(ct == CT - 1))
                # strided write skips each head's ones column
                nc.vector.tensor_add(
                    v4[:, t, :, 0:HD],
                    p.rearrange("p (h e) -> p h e", e=HD),
                    bv_bc.rearrange("p (h e) -> p h e", e=HD))

            # upfront: enough for the front stream to start
            emit_q(0, 0)
            emit_q(0, 1)
            emit_k(0, 0)

            # front-stream chunks (K/Q): due = front iteration they must
            # precede. back-stream chunks (V): due = back iteration.
            fq, vq = [], []
            fq += [(3, emit_k, (0, 1)), (7, emit_k, (0, 2)),
                   (11, emit_k, (0, 3)), (14, emit_q, (0, 2)),
                   (15, emit_q, (0, 3))]
            for j in range(1, 4):
                base = 64 * (j - 1) + 20
                order = [(emit_q, (j, 0)), (emit_q, (j, 1)),
                         (emit_k, (j, 0)), (emit_k, (j, 1)),
                         (emit_k, (j, 2)), (emit_k, (j, 3)),
                         (emit_q, (j, 2)), (emit_q, (j, 3))]
                fq += [(base + 4 * k, fn, a) for k, (fn, a) in enumerate(order)]
            vq += [(t, emit_v, (t,)) for t in range(ST)]
            fq.sort(key=lambda x: x[0])
            fi = {"f": 0, "v": 0}

            def drain_front(t):
                while fi["f"] < len(fq) and fq[fi["f"]][0] <= t:
                    _, fn, a = fq[fi["f"]]
                    fi["f"] += 1
                    fn(*a)

            def drain_back(t):
                while fi["v"] < len(vq) and vq[fi["v"]][0] <= t:
                    _, fn, a = vq[fi["v"]]
                    fi["v"] += 1
                    fn(*a)

            # ---- two-stream attention pipeline ----
            NIT = HPC * 2 * ST        # 256
            ctx_tile = [None]

            def front(t):
                b, st = t // ST, t % ST
                h, ih = b // 2, b % 2
                i0 = ih * 1024
                s_ps = ppool.tile([128, 1024], F32, tag=f"S{t % 2}",
                                  name=f"sc{t}")
                for q2 in range(2):
                    nc.tensor.matmul(
                        s_ps[:, q2 * 512:(q2 + 1) * 512],
                        lhsT=kt_sb[h // 2][:, st * 128:(st + 1) * 128],
                        rhs=qp_sb[h][:, i0 + q2 * 512:i0 + (q2 + 1) * 512],
                        start=True, stop=True)
                e_sb = epool.tile([128, 1024], BF16, tag="e", name=f"e{t}")
                nc.scalar.activation(
                    e_sb, s_ps,
                    mybir.ActivationFunctionType.Exp,
                    bias=mask_sb[:, st:st + 1], scale=0.125)
                return e_sb

            e_ring = {}

            def back(t):
                b, st = t // ST, t % ST
                h, ih = b // 2, b % 2
                i0 = ih * 1024
                if st == 0:
                    ctx_tile[0] = ppool.tile([VW, 1024], F32, tag="C",
                                             name=f"ctx{b}")
                e_sb = e_ring.pop(t)
                for q2 in range(2):
                    nc.tensor.matmul(
                        ctx_tile[0][:, q2 * 512:(q2 + 1) * 512],
                        lhsT=v_sb[:, st, h * VW:(h + 1) * VW],
                        rhs=e_sb[:, q2 * 512:(q2 + 1) * 512],
                        start=(st == 0), stop=(st == ST - 1))
                if st == ST - 1:
                    o_sb = opool.tile([VW, 1024], F32, tag="o", name=f"o{b}")
                    nc.vector.tensor_copy(o_sb, ctx_tile[0])
                    nc.sync.dma_start(
                        out=out[h * VW:(h + 1) * VW, i0:i0 + 1024], in_=o_sb)

            for t in range(NIT + LAG):
                if t < NIT:
                    drain_front(t)
                    e_ring[t] = front(t)
                if t >= LAG:
                    bt = t - LAG
                    drain_back(bt)
                    back(bt)

    nc.compile()
    return nc


def _get_nc():
    if "nc" not in _CACHE:
        _CACHE["nc"] = _build()
    return _CACHE["nc"]


def _in_maps(hidden_states, attention_mask, wq, bq, wk, bk, wv, bv):
    import ml_dtypes

    bf16 = ml_dtypes.bfloat16
    maps = []
    for c in range(8):
        b, g = c // 2, c % 2
        ch0 = g * CH
        maps.append({
            "xt": np.ascontiguousarray(hidden_states[b].T.astype(bf16)),
            "wq_t": np.ascontiguousarray(wq[ch0:ch0 + CH, :].T.astype(bf16)),
            "wk_t": np.ascontiguousarray(wk[ch0:ch0 + CH, :].T.astype(bf16)),
            "wv_t": np.ascontiguousarray(wv[ch0:ch0 + CH, :].T.astype(bf16)),
            "bq": np.ascontiguousarray(bq[ch0:ch0 + CH]),
            "bk": np.ascontiguousarray(bk[ch0:ch0 + CH]),
            "bv": np.ascontiguousarray(bv[ch0:ch0 + CH]),
            "mask": np.ascontiguousarray(attention_mask[b, 0, 0, :]),
        })
    return maps


def _gather(results):
    full = np.empty((B, S, H), np.float32)
    for c in range(8):
        b, g = c // 2, c % 2
        o = results[c]["out"].reshape(HPC, VW, S)
        ctx = o[:, :HD, :] / o[:, HD:HD + 1, :]        # normalize by denom row
        # [h, d, s] -> [s, h*d]
        full[b, :, g * CH:(g + 1) * CH] = ctx.reshape(CH, S).T
    return full


def _run(in_maps, trace=False):
    from concourse.bass_utils import run_bass_kernel_spmd

    nc = _get_nc()
    return run_bass_kernel_spmd(nc, in_maps, list(range(8)), trace=trace)


def _run_results(in_maps):
    """Run on hardware; on a wedged-device error retry in fresh subprocesses
    (the PJRT client cannot recover an unrecoverable exec unit in-process)."""
    try:
        return _run(in_maps).results
    except Exception:
        pass
    import pickle
    import subprocess
    import tempfile

    last = None
    for _ in range(3):
        try:
            with tempfile.TemporaryDirectory() as td:
                fin = os.path.join(td, "in.pkl")
                fout = os.path.join(td, "out.pkl")
                with open(fin, "wb") as f:
                    pickle.dump(in_maps, f)
                code = (
                    "import pickle, sys\n"
                    f"sys.path.insert(0, {_KERNEL_DIR!r})\n"
                    "import kernel\n"
                    f"maps = pickle.load(open({fin!r}, 'rb'))\n"
                    "res = kernel._run(maps)\n"
                    f"pickle.dump(res.results, open({fout!r}, 'wb'))\n"
                )
                subprocess.run([sys.executable, "-c", code], check=True,
                               timeout=1800)
                with open(fout, "rb") as f:
                    return pickle.load(f)
        except Exception as e:
            last = e
    raise last


def kernel(hidden_states, attention_mask, wq, bq, wk, bk, wv, bv):
    args = [np.asarray(a, np.float32) for a in
            (hidden_states, attention_mask, wq, bq, wk, bk, wv, bv)]
    return _gather(_run_results(_in_maps(*args)))


def kernel_profiled(hidden_states, attention_mask, wq, bq, wk, bk, wv, bv):
    """Like kernel() but with NTFF tracing; returns (output, exec_time_ns)."""
    args = [np.asarray(a, np.float32) for a in
            (hidden_states, attention_mask, wq, bq, wk, bk, wv, bv)]
    res = _run(_in_maps(*args), trace=True)
    return _gather(res.results), res.exec_time_ns


# revision 9
# speedup vs baseline: 1.3318x; 1.0139x over previous
"""BertSelfAttention (B=4, S=2048, H=1024, NH=16, HD=64) on 8 Trainium2 NeuronCores.

Sharding: batch (4) x head-group (2) -> 8 cores. Core c handles batch b=c//2 and
heads [g*8, g*8+8) with g=c%2 (output channels [g*512, (g+1)*512)).

v3: bf16 matmul path, software-pipelined two-stream emission.

The kernel sits at a balanced machine point: softmax exp is 33.6M
elements/core on the only engine that can evaluate Exp (ScalarE, 1
elem/cycle/lane @ 1.2 GHz -> ~294us), while the PE streams 720896 matmul
columns (~300us @ 2.4 GHz). Span is therefore set by whichever engine
stalls less, and the whole design is about keeping both >95% busy:

  * All matmul inputs are bf16 (host casts): Fast Weight Load hides
    LDWEIGHTS behind the previous matmul's streaming; DMA and SBUF halve.
  * x^T stays resident in SBUF (4MB bf16), loaded in 8 batched DMAs; the
    4 scalar staging DMAs (mask/biases) are emitted first so nothing
    waits on them. 18 dma_starts total (~1us SWDGE setup each).
  * Two-stream software pipeline over the 256 attention iterations:
    the FRONT stream (scores matmuls + exp) runs ahead; the BACK stream
    (ctx matmuls, V-projection chunks, ctx drains) trails LAG=12
    iterations through a 16-deep bf16 e-tile pool. The ACT engine only
    ever waits on scores; V-projection storms and ctx drains are
    absorbed by the lag.
  * QKV projection is emitted as 1-PSUM-bank chunks (8 matmuls + DVE
    drain): Q/K chunks are due-scheduled into the front stream just
    before the scores that read them, V chunks into the back stream just
    before the ctx that reads them, paced so the PE's per-iteration
    slack absorbs them.
  * PSUM = 8 banks: scores ping-pong 2x[128,1024] (4), one ctx
    accumulator [65,1024] (2), projection chunk ping-pong 2x[128,512] (2).
  * Q is stored per-head zero-padded to 128 partitions so the scores
    contraction is always K=128 against 2-head-packed K tiles; the fused
    ones column in V (65 cols/head) makes the softmax denominator a free
    extra ctx output row; the host divides.

Device emits unnormalized ctxT + denom rows [8*65, 2048] fp32; the host
divides and transposes into [B, S, H].
"""

import os
import sys

if "/opt/trn_rl_repo" not in sys.path:
    sys.path.insert(0, "/opt/trn_rl_repo")

import numpy as np

_KERNEL_DIR = os.path.dirname(os.path.abspath(__file__))

B, S, H = 4, 2048, 1024
NH, HD = 16, 64
HPC = 8          # heads per core
CH = HPC * HD    # 512 output channels per core
CT = H // 128    # 8 contraction tiles
ST = S // 128    # 16 key/token tiles
VW = HD + 1      # 65: v columns + fused ones column
LAG = 12         # back-stream (ctx) lag in iterations
EBUFS = 16       # e-tile pool depth (must be > LAG + 2)

_CACHE = {}


def _build():
    import concourse.bass as bass  # noqa: F401  (registers engine methods)
    import concourse.mybir as mybir
    import concourse.tile as tile
    from concourse import bacc
    from contextlib import ExitStack

    F32 = mybir.dt.float32
    BF16 = mybir.dt.bfloat16

    nc = bacc.Bacc("TRN2", target_bir_lowering=False, debug=True)

    xt = nc.dram_tensor("xt", [H, S], BF16, kind="ExternalInput")        # x_b^T
    wq_t = nc.dram_tensor("wq_t", [H, CH], BF16, kind="ExternalInput")   # wq_c^T
    wk_t = nc.dram_tensor("wk_t", [H, CH], BF16, kind="ExternalInput")
    wv_t = nc.dram_tensor("wv_t", [H, CH], BF16, kind="ExternalInput")
    # bqe/bqo: per-partition Q-bias columns pre-masked for the even/odd
    # head row layout; qm: [:,0]=rows 0:64 one-hot, [:,1]=rows 64:128.
    # Used by the fused zero-pad Q drains (out = psum*mask + bias).
    bqe = nc.dram_tensor("bqe", [128, 4], F32, kind="ExternalInput")
    bqo = nc.dram_tensor("bqo", [128, 4], F32, kind="ExternalInput")
    qm = nc.dram_tensor("qm", [128, 2], F32, kind="ExternalInput")
    bk = nc.dram_tensor("bk", [CH], F32, kind="ExternalInput")
    bv = nc.dram_tensor("bv", [CH], F32, kind="ExternalInput")
    mask = nc.dram_tensor("mask", [S], F32, kind="ExternalInput")
    # unnormalized ctxT + denominator rows, 65 rows per head
    out = nc.dram_tensor("out", [VW * HPC, S], F32, kind="ExternalOutput")

    with tile.TileContext(nc) as tc, nc.allow_low_precision(reason="bf16 attention"):
        with ExitStack() as stk:
            persist = stk.enter_context(tc.tile_pool(name="persist", bufs=1))
            ppool = stk.enter_context(tc.tile_pool(name="pp", bufs=1, space="PSUM"))
            epool = stk.enter_context(tc.tile_pool(name="ep", bufs=EBUFS))
            opool = stk.enter_context(tc.tile_pool(name="op", bufs=3))

            # ---- persistent SBUF tensors ----
            # x^T resident: 4 tiles [128, 8*512], one per query-quarter sq;
            # ct block ct lives at cols [ct*512, (ct+1)*512).
            x_sb = [persist.tile([128, CT * 512], BF16, tag=f"x{sq}",
                                 name=f"x{sq}") for sq in range(4)]
            # weights: one tile per tensor, ct block at cols [ct*512, ...)
            wq_sb = persist.tile([128, CT * 512], BF16, tag="wq")
            wk_sb = persist.tile([128, CT * 512], BF16, tag="wk")
            wv_sb = persist.tile([128, CT * 512], BF16, tag="wv")
            # Q per head, zero-padded to 128 partitions (head h owns rows
            # (h%2)*64:(h%2)*64+64; the other 64 rows are zeros).
            qp_sb = [persist.tile([128, S], BF16, tag=f"qp{h}", name=f"qp{h}")
                     for h in range(HPC)]
            kt_sb = [persist.tile([128, S], BF16, tag=f"kt{j}", name=f"kt{j}")
                     for j in range(4)]
            v_sb = persist.tile([128, ST, VW * HPC], BF16, tag="v")
            mask_sb = persist.tile([128, ST], F32, tag="mask")
            bqe_sb = persist.tile([128, 4], F32, tag="bqe")
            bqo_sb = persist.tile([128, 4], F32, tag="bqo")
            qm_sb = persist.tile([128, 2], F32, tag="qm")
            bkp = persist.tile([128, 4], F32, tag="bkp")
            bv_bc = persist.tile([128, CH], F32, tag="bv_bc")

            # ones columns of v (the zero-pad of Q is fused into its drains)
            v4 = v_sb.rearrange("p t (h e) -> p t h e", e=VW)
            nc.vector.memset(v4[:, :, :, HD], 1.0)

            # ---- staging DMAs: scalars first, then batched x/w ----
            nc.sync.dma_start(out=mask_sb, in_=mask.rearrange("(t p) -> p t", p=128))
            nc.sync.dma_start(out=bqe_sb, in_=bqe[:, :])
            nc.sync.dma_start(out=bqo_sb, in_=bqo[:, :])
            nc.sync.dma_start(out=qm_sb, in_=qm[:, :])
            nc.sync.dma_start(out=bkp, in_=bk.rearrange("(j p) -> p j", p=128))
            nc.sync.dma_start(
                out=bv_bc,
                in_=bass.AP(tensor=bv, offset=0, ap=[[0, 128], [1, CH]]))

            # xt [H=(c p), S=(q s)] -> [p, c, q, s]
            xt_r = xt.rearrange("(c p) (q s) -> p c q s", p=128, s=512)
            w_r = {t.name: t.rearrange("(c p) j -> p c j", p=128)
                   for t in (wq_t, wk_t, wv_t)}

            def dma_w(w_sb, dram_name, quad):
                cs = slice(4 * quad, 4 * quad + 4)
                nc.sync.dma_start(
                    out=w_sb[:, quad * 2048:(quad + 1) * 2048].rearrange(
                        "p (c j) -> p c j", j=512),
                    in_=w_r[dram_name][:, cs, :])

            def dma_x(sq, quad):
                cs = slice(4 * quad, 4 * quad + 4)
                nc.sync.dma_start(
                    out=x_sb[sq][:, quad * 2048:(quad + 1) * 2048].rearrange(
                        "p (c s) -> p c s", s=512),
                    in_=xt_r[:, cs, sq, :])

            dma_w(wq_sb, "wq_t", 0)
            dma_w(wq_sb, "wq_t", 1)
            dma_x(0, 0)
            dma_x(0, 1)
            dma_w(wk_sb, "wk_t", 0)
            dma_w(wk_sb, "wk_t", 1)
            dma_w(wv_sb, "wv_t", 0)
            dma_w(wv_sb, "wv_t", 1)
            for sq in range(1, 4):
                dma_x(sq, 0)
                dma_x(sq, 1)

            # ---- projection chunks (1 PSUM bank each) ----
            pstate = {"n": 0}

            def pchunk(nm):
                t = ppool.tile([128, 512], F32, tag=f"P{pstate['n'] % 2}",
                               name=nm)
                pstate["n"] += 1
                return t

            def emit_q(j, sq):
                p = pchunk(f"pq{j}_{sq}")
                for ct in range(CT):
                    nc.tensor.matmul(
                        p, lhsT=wq_sb[:, ct * 512 + j * 128:ct * 512 + (j + 1) * 128],
                        rhs=x_sb[sq][:, ct * 512:(ct + 1) * 512],
                        start=(ct == 0), stop=(ct == CT - 1))
                h0, h1 = 2 * j, 2 * j + 1
                qr = slice(sq * 512, (sq + 1) * 512)
                # fused drain + zero-pad: qp = psum*rowmask + masked-bias
                nc.vector.tensor_scalar(
                    qp_sb[h0][:, qr], p, qm_sb[:, 0:1], bqe_sb[:, j:j + 1],
                    mybir.AluOpType.mult, mybir.AluOpType.add)
                nc.vector.tensor_scalar(
                    qp_sb[h1][:, qr], p, qm_sb[:, 1:2], bqo_sb[:, j:j + 1],
                    mybir.AluOpType.mult, mybir.AluOpType.add)

            def emit_k(j, sq):
                p = pchunk(f"pk{j}_{sq}")
                for ct in range(CT):
                    nc.tensor.matmul(
                        p, lhsT=wk_sb[:, ct * 512 + j * 128:ct * 512 + (j + 1) * 128],
                        rhs=x_sb[sq][:, ct * 512:(ct + 1) * 512],
                        start=(ct == 0), stop=(ct == CT - 1))
                nc.vector.tensor_scalar_add(
                    kt_sb[j][:, sq * 512:(sq + 1) * 512], p, bkp[:, j:j + 1])

            def emit_v(t):
                p = pchunk(f"pv{t}")
                sq, c0 = t // 4, (t % 4) * 128
                for ct in range(CT):
                    nc.tensor.matmul(
                        p, lhsT=x_sb[sq][:, ct * 512 + c0:ct * 512 + c0 + 128],
                        rhs=wv_sb[:, ct * 512:(ct + 1) * 512],
                        start=(ct == 0), stop=(ct == CT - 1))
                # strided write skips each head's ones column
                nc.vector.tensor_add(
                    v4[:, t, :, 0:HD],
                    p.rearrange("p (h e) -> p h e", e=HD),
                    bv_bc.rearrange("p (h e) -> p h e", e=HD))

            # upfront: enough for the front stream to start
            emit_q(0, 0)
            emit_q(0, 1)
            emit_k(0, 0)

            # front-stream chunks (K/Q): due = front iteration they must
            # precede. back-stream chunks (V): due = back iteration.
            fq, vq = [], []
            fq += [(3, emit_k, (0, 1)), (7, emit_k, (0, 2)),
                   (11, emit_k, (0, 3)), (14, emit_q, (0, 2)),
                   (15, emit_q, (0, 3))]
            # j1..j3 spread wide (and pushed late) so the PE always has
            # background work, including near the kernel tail.
            windows = {1: (20, 6), 2: (70, 8), 3: (134, 10)}
            for j in range(1, 4):
                base, step = windows[j]
                order = [(emit_q, (j, 0)), (emit_q, (j, 1)),
                         (emit_k, (j, 0)), (emit_k, (j, 1)),
                         (emit_k, (j, 2)), (emit_k, (j, 3)),
                         (emit_q, (j, 2)), (emit_q, (j, 3))]
                fq += [(base + step * k, fn, a) for k, (fn, a) in enumerate(order)]
            vq += [(t, emit_v, (t,)) for t in range(ST)]
            fq.sort(key=lambda x: x[0])
            fi = {"f": 0, "v": 0}

            def drain_front(t):
                while fi["f"] < len(fq) and fq[fi["f"]][0] <= t:
                    _, fn, a = fq[fi["f"]]
                    fi["f"] += 1
                    fn(*a)

            def drain_back(t):
                while fi["v"] < len(vq) and vq[fi["v"]][0] <= t:
                    _, fn, a = vq[fi["v"]]
                    fi["v"] += 1
                    fn(*a)

            # ---- two-stream attention pipeline ----
            NIT = HPC * 2 * ST        # 256
            ctx_tile = [None]

            def front(t):
                b, st = t // ST, t % ST
                h, ih = b // 2, b % 2
                i0 = ih * 1024
                s_ps = ppool.tile([128, 1024], F32, tag=f"S{t % 2}",
                                  name=f"sc{t}")
                for q2 in range(2):
                    nc.tensor.matmul(
                        s_ps[:, q2 * 512:(q2 + 1) * 512],
                        lhsT=kt_sb[h // 2][:, st * 128:(st + 1) * 128],
                        rhs=qp_sb[h][:, i0 + q2 * 512:i0 + (q2 + 1) * 512],
                        start=True, stop=True)
                e_sb = epool.tile([128, 1024], BF16, tag="e", name=f"e{t}")
                nc.scalar.activation(
                    e_sb, s_ps,
                    mybir.ActivationFunctionType.Exp,
                    bias=mask_sb[:, st:st + 1], scale=0.125)
                return e_sb

            e_ring = {}

            def back(t):
                b, st = t // ST, t % ST
                h, ih = b // 2, b % 2
                i0 = ih * 1024
                if st == 0:
                    ctx_tile[0] = ppool.tile([VW, 1024], F32, tag="C",
                                             name=f"ctx{b}")
                e_sb = e_ring.pop(t)
                for q2 in range(2):
                    nc.tensor.matmul(
                        ctx_tile[0][:, q2 * 512:(q2 + 1) * 512],
                        lhsT=v_sb[:, st, h * VW:(h + 1) * VW],
                        rhs=e_sb[:, q2 * 512:(q2 + 1) * 512],
                        start=(st == 0), stop=(st == ST - 1))
                if st == ST - 1:
                    o_sb = opool.tile([VW, 1024], F32, tag="o", name=f"o{b}")
                    nc.vector.tensor_copy(o_sb, ctx_tile[0])
                    nc.sync.dma_start(
                        out=out[h * VW:(h + 1) * VW, i0:i0 + 1024], in_=o_sb)

            for t in range(NIT + LAG):
                if t < NIT:
                    drain_front(t)
                    e_ring[t] = front(t)
                if t >= LAG:
                    bt = t - LAG
                    drain_back(bt)
                    back(bt)

    nc.compile()
    return nc


def _get_nc():
    if "nc" not in _CACHE:
        _CACHE["nc"] = _build()
    return _CACHE["nc"]


def _in_maps(hidden_states, attention_mask, wq, bq, wk, bk, wv, bv):
    import ml_dtypes

    bf16 = ml_dtypes.bfloat16
    qm = np.zeros((128, 2), np.float32)
    qm[0:64, 0] = 1.0        # head-even owns rows 0:64
    qm[64:128, 1] = 1.0      # head-odd owns rows 64:128
    maps = []
    for c in range(8):
        b, g = c // 2, c % 2
        ch0 = g * CH
        bq_sb = np.ascontiguousarray(
            bq[ch0:ch0 + CH].reshape(4, 128).T.astype(np.float32))
        bqe = bq_sb * qm[:, 0:1]
        bqo = bq_sb * qm[:, 1:2]
        maps.append({
            "xt": np.ascontiguousarray(hidden_states[b].T.astype(bf16)),
            "wq_t": np.ascontiguousarray(wq[ch0:ch0 + CH, :].T.astype(bf16)),
            "wk_t": np.ascontiguousarray(wk[ch0:ch0 + CH, :].T.astype(bf16)),
            "wv_t": np.ascontiguousarray(wv[ch0:ch0 + CH, :].T.astype(bf16)),
            "bqe": bqe,
            "bqo": bqo,
            "qm": qm,
            "bk": np.ascontiguousarray(bk[ch0:ch0 + CH]),
            "bv": np.ascontiguousarray(bv[ch0:ch0 + CH]),
            "mask": np.ascontiguousarray(attention_mask[b, 0, 0, :]),
        })
    return maps


def _gather(results):
    full = np.empty((B, S, H), np.float32)
    for c in range(8):
        b, g = c // 2, c % 2
        o = results[c]["out"].reshape(HPC, VW, S)
        ctx = o[:, :HD, :] / o[:, HD:HD + 1, :]        # normalize by denom row
        # [h, d, s] -> [s, h*d]
        full[b, :, g * CH:(g + 1) * CH] = ctx.reshape(CH, S).T
    return full


def _run(in_maps, trace=False):
    from concourse.bass_utils import run_bass_kernel_spmd

    nc = _get_nc()
    return run_bass_kernel_spmd(nc, in_maps, list(range(8)), trace=trace)


def _run_results(in_maps):
    """Run on hardware; on a wedged-device error retry in fresh subprocesses
    (the PJRT client cannot recover an unrecoverable exec unit in-process)."""
    try:
        return _run(in_maps).results
    except Exception:
        pass
    import pickle
    import subprocess
    import tempfile

    last = None
    for _ in range(3):
        try:
            with tempfile.TemporaryDirectory() as td:
                fin = os.path.join(td, "in.pkl")
                fout = os.path.join(td, "out.pkl")
                with open(fin, "wb") as f:
                    pickle.dump(in_maps, f)
                code = (
                    "import pickle, sys\n"
                    f"sys.path.insert(0, {_KERNEL_DIR!r})\n"
                    "import kernel\n"
                    f"maps = pickle.load(open({fin!r}, 'rb'))\n"
                    "res = kernel._run(maps)\n"
                    f"pickle.dump(res.results, open({fout!r}, 'wb'))\n"
                )
                subprocess.run([sys.executable, "-c", code], check=True,
                               timeout=1800)
                with open(fout, "rb") as f:
                    return pickle.load(f)
        except Exception as e:
            last = e
    raise last


def kernel(hidden_states, attention_mask, wq, bq, wk, bk, wv, bv):
    args = [np.asarray(a, np.float32) for a in
            (hidden_states, attention_mask, wq, bq, wk, bk, wv, bv)]
    return _gather(_run_results(_in_maps(*args)))


def kernel_profiled(hidden_states, attention_mask, wq, bq, wk, bk, wv, bv):
    """Like kernel() but with NTFF tracing; returns (output, exec_time_ns)."""
    args = [np.asarray(a, np.float32) for a in
            (hidden_states, attention_mask, wq, bq, wk, bk, wv, bv)]
    res = _run(_in_maps(*args), trace=True)
    return _gather(res.results), res.exec_time_ns


# revision 10
# speedup vs baseline: 1.3466x; 1.0111x over previous
"""BertSelfAttention (B=4, S=2048, H=1024, NH=16, HD=64) on 8 Trainium2 NeuronCores.

Sharding: batch (4) x head-group (2) -> 8 cores. Core c handles batch b=c//2 and
heads [g*8, g*8+8) with g=c%2 (output channels [g*512, (g+1)*512)).

v5: row-tiled concurrent 2-head scores, bf16 path, two-stream pipeline.

Engine budget per core: softmax exp is 33.6M elements on the only engine
that evaluates Exp (ScalarE, 1 elem/cycle/lane @ 1.2GHz -> ~268us of ACT
work in 256 [128,1024] tiles); the PE streams ~590k matmul columns
(~246us @ 2.4GHz). ACT is the critical path; the design keeps it >95%
busy from ~10us onward:

  * Scores use PE row-tiling: the two heads of a K-channel pair sit on
    SBUF partitions 0:64 / 64:128 (K tiles pack them the same way), so
    the two K=64 score matmuls occupy disjoint PE row-groups and run
    CONCURRENTLY (tile_position auto-derived from base partitions) --
    both heads' scores for a 512-query block in ~one matmul time. This
    also removes the v1-v4 zero-padded-Q trick entirely.
  * One [128,1024] exp per (j, qblock, st) covers both heads (same keys
    on partitions -> same per-partition mask bias).
  * All matmul inputs bf16 (host casts); x^T and weights are host-packed
    into SBUF-layout contiguous arrays so staging DMAs move 4KB lines.
  * Two-stream software pipeline over 256 iterations: FRONT (scores+exp)
    runs ahead; BACK (ctx matmuls, V-projection chunks, drains) trails
    LAG=12 iterations through a 16-deep bf16 e-tile pool.
  * QKV projection is emitted as 1-PSUM-bank chunks due-scheduled into
    the PE's slack (~0.4us/iteration), spread across the whole kernel.
  * PSUM = 8 banks: scores ping-pong 2x[128,1024] (4), ctx accumulator
    [65,1024] = both heads' [65,512] blocks (2), chunk ping-pong (2).
  * The fused ones column in V (65 cols/head) makes the softmax
    denominator a free extra ctx output row; the host divides.

Device emits unnormalized ctxT + denom rows [8*65, 2048] fp32; the host
divides and transposes into [B, S, H].
"""

import os
import sys

if "/opt/trn_rl_repo" not in sys.path:
    sys.path.insert(0, "/opt/trn_rl_repo")

import numpy as np

_KERNEL_DIR = os.path.dirname(os.path.abspath(__file__))

B, S, H = 4, 2048, 1024
NH, HD = 16, 64
HPC = 8          # heads per core
CH = HPC * HD    # 512 output channels per core
CT = H // 128    # 8 contraction tiles
ST = S // 128    # 16 key/token tiles
VW = HD + 1      # 65: v columns + fused ones column
LAG = 12         # back-stream (ctx) lag in iterations
EBUFS = 16       # e-tile pool depth (must be > LAG + 2)

_CACHE = {}


def _build():
    import concourse.bass as bass  # noqa: F401  (registers engine methods)
    import concourse.mybir as mybir
    import concourse.tile as tile
    from concourse import bacc
    from contextlib import ExitStack

    F32 = mybir.dt.float32
    BF16 = mybir.dt.bfloat16

    nc = bacc.Bacc("TRN2", target_bir_lowering=False, debug=True)

    # host-packed layouts (see _in_maps): per partition p,
    #   xt[p, (sq*8+ct)*512 + s] = x^T[128*ct+p, 512*sq+s]
    #   w*[p, ct*512 + c]        = W^T[128*ct+p, c]
    xt = nc.dram_tensor("xt", [128, 4 * CT * 512], BF16, kind="ExternalInput")
    wq_t = nc.dram_tensor("wq_t", [128, CT * 512], BF16, kind="ExternalInput")
    wk_t = nc.dram_tensor("wk_t", [128, CT * 512], BF16, kind="ExternalInput")
    wv_t = nc.dram_tensor("wv_t", [128, CT * 512], BF16, kind="ExternalInput")
    bq = nc.dram_tensor("bq", [CH], F32, kind="ExternalInput")
    bk = nc.dram_tensor("bk", [CH], F32, kind="ExternalInput")
    bv = nc.dram_tensor("bv", [CH], F32, kind="ExternalInput")
    mask = nc.dram_tensor("mask", [S], F32, kind="ExternalInput")
    # unnormalized ctxT + denominator rows, 65 rows per head
    out = nc.dram_tensor("out", [VW * HPC, S], F32, kind="ExternalOutput")

    with tile.TileContext(nc) as tc, nc.allow_low_precision(reason="bf16 attention"):
        with ExitStack() as stk:
            persist = stk.enter_context(tc.tile_pool(name="persist", bufs=1))
            ppool = stk.enter_context(tc.tile_pool(name="pp", bufs=1, space="PSUM"))
            epool = stk.enter_context(tc.tile_pool(name="ep", bufs=EBUFS))
            opool = stk.enter_context(tc.tile_pool(name="op", bufs=3))

            # ---- persistent SBUF tensors ----
            # x: 8 tiles [(sq, half)] of [128, 2048] (ct-quad per tile)
            x_sb = [[persist.tile([128, 2048], BF16, tag=f"x{sq}_{hf}",
                                  name=f"x{sq}_{hf}") for hf in range(2)]
                    for sq in range(4)]
            w_sb = {}
            for nm in ("wq", "wk", "wv"):
                w_sb[nm] = [persist.tile([128, 2048], BF16, tag=f"{nm}{hf}",
                                         name=f"{nm}{hf}") for hf in range(2)]
            # Q per head-pair j: rows 0:64 head 2j, rows 64:128 head 2j+1
            qp_sb = [persist.tile([128, S], BF16, tag=f"qp{j}", name=f"qp{j}")
                     for j in range(4)]
            kt_sb = [persist.tile([128, S], BF16, tag=f"kt{j}", name=f"kt{j}")
                     for j in range(4)]
            v_sb = persist.tile([128, ST, VW * HPC], BF16, tag="v")
            mask_sb = persist.tile([128, ST], F32, tag="mask")
            bqp = persist.tile([128, 4], F32, tag="bqp")
            bkp = persist.tile([128, 4], F32, tag="bkp")
            bv_bc = persist.tile([128, CH], F32, tag="bv_bc")

            # ones columns of v
            v4 = v_sb.rearrange("p t (h e) -> p t h e", e=VW)
            nc.vector.memset(v4[:, :, :, HD], 1.0)

            # ---- staging DMAs: scalars first, then packed x/w ----
            nc.sync.dma_start(out=mask_sb, in_=mask.rearrange("(t p) -> p t", p=128))
            nc.sync.dma_start(out=bqp, in_=bq.rearrange("(j p) -> p j", p=128))
            nc.sync.dma_start(out=bkp, in_=bk.rearrange("(j p) -> p j", p=128))
            nc.sync.dma_start(
                out=bv_bc,
                in_=bass.AP(tensor=bv, offset=0, ap=[[0, 128], [1, CH]]))

            def dma_w(nm, dram, hf):
                nc.sync.dma_start(out=w_sb[nm][hf],
                                  in_=dram[:, hf * 2048:(hf + 1) * 2048])

            def dma_x(sq, hf):
                o = (sq * 8 + hf * 4) * 512
                nc.sync.dma_start(out=x_sb[sq][hf], in_=xt[:, o:o + 2048])

            dma_w("wq", wq_t, 0)
            dma_w("wq", wq_t, 1)
            dma_x(0, 0)
            dma_x(0, 1)
            dma_w("wk", wk_t, 0)
            dma_w("wk", wk_t, 1)
            dma_w("wv", wv_t, 0)
            dma_w("wv", wv_t, 1)
            for sq in range(1, 4):
                dma_x(sq, 0)
                dma_x(sq, 1)

            # ---- projection chunks (1 PSUM bank each) ----
            pstate = {"n": 0}

            def pchunk(nm):
                t = ppool.tile([128, 512], F32, tag=f"P{pstate['n'] % 2}",
                               name=nm)
                pstate["n"] += 1
                return t

            def wsl(nm, ct, j=None):
                t = w_sb[nm][ct // 4]
                o = (ct % 4) * 512
                if j is None:
                    return t[:, o:o + 512]
                return t[:, o + j * 128:o + (j + 1) * 128]

            def xsl(sq, ct, c0=0, w=512):
                t = x_sb[sq][ct // 4]
                o = (ct % 4) * 512 + c0
                return t[:, o:o + w]

            def emit_q(j, sq):
                p = pchunk(f"pq{j}_{sq}")
                for ct in range(CT):
                    nc.tensor.matmul(p, lhsT=wsl("wq", ct, j), rhs=xsl(sq, ct),
                                     start=(ct == 0), stop=(ct == CT - 1))
                nc.vector.tensor_scalar_add(
                    qp_sb[j][:, sq * 512:(sq + 1) * 512], p, bqp[:, j:j + 1])

            def emit_k(j, sq):
                p = pchunk(f"pk{j}_{sq}")
                for ct in range(CT):
                    nc.tensor.matmul(p, lhsT=wsl("wk", ct, j), rhs=xsl(sq, ct),
                                     start=(ct == 0), stop=(ct == CT - 1))
                nc.vector.tensor_scalar_add(
                    kt_sb[j][:, sq * 512:(sq + 1) * 512], p, bkp[:, j:j + 1])

            def emit_v(t):
                p = pchunk(f"pv{t}")
                sq, c0 = t // 4, (t % 4) * 128
                for ct in range(CT):
                    nc.tensor.matmul(p, lhsT=xsl(sq, ct, c0, 128),
                                     rhs=wsl("wv", ct),
                                     start=(ct == 0), stop=(ct == CT - 1))
                nc.vector.tensor_add(
                    v4[:, t, :, 0:HD],
                    p.rearrange("p (h e) -> p h e", e=HD),
                    bv_bc.rearrange("p (h e) -> p h e", e=HD))

            # upfront: first front iteration needs Q(0,0) and K(0,0)
            emit_q(0, 0)
            emit_k(0, 0)

            # front-stream chunks: due = front iteration they must precede.
            # Blocks are (j, qblock ih2): iter t = 64j + 16*ih2 + st.
            # K(j,q) first used at 64j+4q; Q(j,ih2) at 64j+16*ih2.
            fq = [(3, emit_k, (0, 1)), (7, emit_k, (0, 2)),
                  (11, emit_k, (0, 3)), (13, emit_q, (0, 1)),
                  (26, emit_q, (0, 2)), (40, emit_q, (0, 3)),
                  (50, emit_q, (1, 0)), (56, emit_k, (1, 0)),
                  (62, emit_k, (1, 1)), (68, emit_k, (1, 2)),
                  (74, emit_k, (1, 3)), (78, emit_q, (1, 1)),
                  (88, emit_q, (1, 2)), (100, emit_q, (1, 3)),
                  (110, emit_q, (2, 0)), (116, emit_k, (2, 0)),
                  (122, emit_k, (2, 1)), (128, emit_k, (2, 2)),
                  (134, emit_k, (2, 3)), (140, emit_q, (2, 1)),
                  (150, emit_q, (2, 2)), (160, emit_q, (2, 3)),
                  (170, emit_q, (3, 0)), (176, emit_k, (3, 0)),
                  (182, emit_k, (3, 1)), (188, emit_k, (3, 2)),
                  (194, emit_k, (3, 3)), (200, emit_q, (3, 1)),
                  (210, emit_q, (3, 2)), (220, emit_q, (3, 3))]
            vq = [(t, emit_v, (t,)) for t in range(ST)]
            fi = {"f": 0, "v": 0}

            def drain_front(t):
                while fi["f"] < len(fq) and fq[fi["f"]][0] <= t:
                    _, fn, a = fq[fi["f"]]
                    fi["f"] += 1
                    fn(*a)

            def drain_back(t):
                while fi["v"] < len(vq) and vq[fi["v"]][0] <= t:
                    _, fn, a = vq[fi["v"]]
                    fi["v"] += 1
                    fn(*a)

            # ---- two-stream attention pipeline ----
            NIT = HPC * 2 * ST        # 256
            ctx_tile = [None]

            def front(t):
                j, ih2, st = t // 64, (t // 16) % 4, t % 16
                qr = slice(ih2 * 512, (ih2 + 1) * 512)
                kr = slice(st * 128, (st + 1) * 128)
                s_ps = ppool.tile([128, 1024], F32, tag=f"S{t % 2}",
                                  name=f"sc{t}")
                # two K=64 matmuls on disjoint PE row-groups -> concurrent
                nc.tensor.matmul(s_ps[:, 0:512], lhsT=kt_sb[j][0:64, kr],
                                 rhs=qp_sb[j][0:64, qr], start=True, stop=True)
                nc.tensor.matmul(s_ps[:, 512:1024], lhsT=kt_sb[j][64:128, kr],
                                 rhs=qp_sb[j][64:128, qr], start=True, stop=True)
                e_sb = epool.tile([128, 1024], BF16, tag="e", name=f"e{t}")
                nc.scalar.activation(
                    e_sb, s_ps,
                    mybir.ActivationFunctionType.Exp,
                    bias=mask_sb[:, st:st + 1], scale=0.125)
                return e_sb

            e_ring = {}

            def back(t):
                j, ih2, st = t // 64, (t // 16) % 4, t % 16
                h0, h1 = 2 * j, 2 * j + 1
                if st == 0:
                    ctx_tile[0] = ppool.tile([VW, 1024], F32, tag="C",
                                             name=f"ctx{t // 16}")
                ctx = ctx_tile[0]
                e_sb = e_ring.pop(t)
                nc.tensor.matmul(
                    ctx[:, 0:512], lhsT=v_sb[:, st, h0 * VW:(h0 + 1) * VW],
                    rhs=e_sb[:, 0:512], start=(st == 0), stop=(st == ST - 1))
                nc.tensor.matmul(
                    ctx[:, 512:1024], lhsT=v_sb[:, st, h1 * VW:(h1 + 1) * VW],
                    rhs=e_sb[:, 512:1024], start=(st == 0), stop=(st == ST - 1))
                if st == ST - 1:
                    o_sb = opool.tile([VW, 1024], F32, tag="o",
                                      name=f"o{t // 16}")
                    nc.vector.tensor_copy(o_sb, ctx)
                    qr = slice(ih2 * 512, (ih2 + 1) * 512)
                    nc.sync.dma_start(
                        out=out[h0 * VW:(h0 + 1) * VW, qr], in_=o_sb[:, 0:512])
                    nc.sync.dma_start(
                        out=out[h1 * VW:(h1 + 1) * VW, qr],
                        in_=o_sb[:, 512:1024])

            for t in range(NIT + LAG):
                if t < NIT:
                    drain_front(t)
                    e_ring[t] = front(t)
                if t >= LAG:
                    bt = t - LAG
                    drain_back(bt)
                    back(bt)

    nc.compile()
    return nc


def _get_nc():
    if "nc" not in _CACHE:
        _CACHE["nc"] = _build()
    return _CACHE["nc"]


def _in_maps(hidden_states, attention_mask, wq, bq, wk, bk, wv, bv):
    import ml_dtypes

    bf16 = ml_dtypes.bfloat16

    def pack_w(w):                      # [H, CH] -> [128, CT*512]
        return np.ascontiguousarray(
            w.reshape(CT, 128, CH).transpose(1, 0, 2).reshape(128, CT * CH))

    maps = []
    for c in range(8):
        b, g = c // 2, c % 2
        ch0 = g * CH
        xt_arr = hidden_states[b].T.astype(bf16)          # [H, S]
        xt_p = np.ascontiguousarray(
            xt_arr.reshape(CT, 128, 4, 512).transpose(1, 2, 0, 3)
            .reshape(128, 4 * CT * 512))
        maps.append({
            "xt": xt_p,
            "wq_t": pack_w(wq[ch0:ch0 + CH, :].T.astype(bf16)),
            "wk_t": pack_w(wk[ch0:ch0 + CH, :].T.astype(bf16)),
            "wv_t": pack_w(wv[ch0:ch0 + CH, :].T.astype(bf16)),
            "bq": np.ascontiguousarray(bq[ch0:ch0 + CH]),
            "bk": np.ascontiguousarray(bk[ch0:ch0 + CH]),
            "bv": np.ascontiguousarray(bv[ch0:ch0 + CH]),
            "mask": np.ascontiguousarray(attention_mask[b, 0, 0, :]),
        })
    return maps


def _gather(results):
    full = np.empty((B, S, H), np.float32)
    for c in range(8):
        b, g = c // 2, c % 2
        o = results[c]["out"].reshape(HPC, VW, S)
        ctx = o[:, :HD, :] / o[:, HD:HD + 1, :]        # normalize by denom row
        # [h, d, s] -> [s, h*d]
        full[b, :, g * CH:(g + 1) * CH] = ctx.reshape(CH, S).T
    return full


def _run(in_maps, trace=False):
    from concourse.bass_utils import run_bass_kernel_spmd

    nc = _get_nc()
    return run_bass_kernel_spmd(nc, in_maps, list(range(8)), trace=trace)


def _run_results(in_maps):
    """Run on hardware; on a wedged-device error retry in fresh subprocesses
    (the PJRT client cannot recover an unrecoverable exec unit in-process)."""
    try:
        return _run(in_maps).results
    except Exception:
        pass
    import pickle
    import subprocess
    import tempfile

    last = None
    for _ in range(3):
        try:
            with tempfile.TemporaryDirectory() as td:
                fin = os.path.join(td, "in.pkl")
                fout = os.path.join(td, "out.pkl")
                with open(fin, "wb") as f:
                    pickle.dump(in_maps, f)
                code = (
                    "import pickle, sys\n"
                    f"sys.path.insert(0, {_KERNEL_DIR!r})\n"
                    "import kernel\n"
                    f"maps = pickle.load(open({fin!r}, 'rb'))\n"
                    "res = kernel._run(maps)\n"
                    f"pickle.dump(res.results, open({fout!r}, 'wb'))\n"
                )
                subprocess.run([sys.executable, "-c", code], check=True,
                               timeout=1800)
                with open(fout, "rb") as f:
                    return pickle.load(f)
        except Exception as e:
            last = e
    raise last


def kernel(hidden_states, attention_mask, wq, bq, wk, bk, wv, bv):
    args = [np.asarray(a, np.float32) for a in
            (hidden_states, attention_mask, wq, bq, wk, bk, wv, bv)]
    return _gather(_run_results(_in_maps(*args)))


def kernel_profiled(hidden_states, attention_mask, wq, bq, wk, bk, wv, bv):
    """Like kernel() but with NTFF tracing; returns (output, exec_time_ns)."""
    args = [np.asarray(a, np.float32) for a in
            (hidden_states, attention_mask, wq, bq, wk, bk, wv, bv)]
    res = _run(_in_maps(*args), trace=True)
    return _gather(res.results), res.exec_time_ns
